# revision 27
# baseline (speedup 1.0000x reference)
"""LinearSelfAttention (elu+1 linear attention) Trainium2 Bass kernel.

Full inputs -> full output. Shards the 32768 tokens (B=4 x N=8192) across 8
NeuronCores as (batch, seq-half); the small kv / k-sum statistics are
all-reduced between the two cores sharing a batch. Weights are replicated.

v2 layout (vs baseline): engineered for tensor-engine continuity (the PE
p-state ramps to 2.4GHz only after ~3us of gapless execution) and for a
hidden AllReduce:

  phase 1 (chunks 0..7): load x chunk [512 tok] (split DMAs so work
    starts on the first half), PE-transpose to x' (feature-major, fp32r
    transposes: 1.5 cyc/row), k/v projection (token-major), elu+1 on k
    via scalar-Exp + scalar-Relu + DVE min/add combine (Exp/Relu/Copy
    live in one act table -> zero table reloads; xt/v PSUM->SBUF copies
    ride the scalar engine as Copy activations), kv|ksum accumulated per
    head-pair with an overlapping-stride rhs (N=260 >= 256 keeps fp32r
    at 1 cyc/row, and the "padding" is just the neighbor pair's data).
    Chunks 0..2 also compute q' (feature-major) inline.
  AllReduce: only the diagonal kv blocks + ksum ship (133KB not 266KB);
    chunks 3..7 defer their q' work until after the AR is dispatched so
    the PE keeps streaming through the collective; the post-AR
    block-diag builds run on the (then idle) vector engine, ksb first
    since the denominator matmul needs it.
  phase 2 (chunks 0..7): block-diagonal kv / ksum-broadcast matmuls give
    out'[e,n] and den[e,n] back-to-back; z = 1/den via
    reciprocal_approx_fast (single custom-DVE op, ~18 bits -- DVE divide
    is not in the V3 ISA and plain reciprocal is 5x slower); ost = out'*z
    overwrites the dead qp slot to save SBUF; y = ost.T @ Wout with bout
    added during the PSUM->SBUF copy (vector tensor_add against a
    partition-broadcast bias).

Engine budget is contention-aware: concurrent DVE/ACT traffic slows PE
matmul streaming by 30-55% (SBUF/PSUM port contention), so elementwise
work is spread thin across scalar/vector/gpsimd and kept off the
matmul-critical chain wherever possible.

All matmul operands are float32r (fp22-rounded fp32) - 1 cyc/row for
free size >= 256, ~1e-4 relative error. fp32r matmuls require even N
and outputs at base partition 0 (walrus ISA constraints).
"""

import numpy as np

import concourse.bass as bass
import concourse.bacc as bacc
import concourse.mybir as mybir
import concourse.tile as tile
from concourse.bass_utils import run_bass_kernel_spmd

B, N, D, H, HD = 4, 8192, 512, 8, 64
NCORES = 8
T = B * N // NCORES          # 4096 tokens per core
NT = 512                     # tokens per chunk
NCH = T // NT                # 8 chunks
DEFER = 5                    # chunks NCH-DEFER..NCH-1 defer q' past the AR
VW = 130                     # per-pair rhs stride: [v_2c|v_2c+1 (128) | 1 | pad]
VTOT = 5 * VW                # 650: pair c reads cols c*130 .. c*130+260
F32 = mybir.dt.float32
F32R = mybir.dt.float32r
AF = mybir.ActivationFunctionType
OP = mybir.AluOpType

REPLICA_GROUPS = [[0, 1], [2, 3], [4, 5], [6, 7]]


def _r(ap):
    return ap.bitcast(F32R)


def _build_kernel(tc, nc, x_d, wqkv_d, wout_d, bout_d, ident_d, y_d):
    with (
        tc.tile_pool(name="consts", bufs=1) as consts,
        tc.tile_pool(name="persist", bufs=1) as persist,
        tc.tile_pool(name="dram", bufs=1, space="DRAM") as dram,
    ):
        # ---------------- constants (DMA priority: ident, x0 ... weights) ---
        ident_sb = consts.tile([128, 128], F32R)
        nc.sync.dma_start(out=ident_sb, in_=_r(ident_d))

        # x chunk pool hoisted so chunk 0's load is issued before weights
        wkv_sb = consts.tile([128, 4, 2 * D], F32R)   # Wqkv cols D:3D
        wq_sb = consts.tile([128, 4, D], F32R)        # Wqkv cols 0:D
        wout_sb = consts.tile([128, 4, D], F32R)
        bout_sb = consts.tile([1, D], F32)
        bout_full = consts.tile([128, D], F32)
        ones_col = consts.tile([128, HD], F32R)

        # persistent state
        qp_sb = persist.tile([128, 4, T], F32R)       # q' (feature-major)
        xt_def = persist.tile([128, DEFER, 4, NT], F32R)  # x' for deferred q'
        cc_sb = persist.tile([128, 4, VW], F32)       # kv|ksum accumulator
        cmp_sb = persist.tile([128, 4, HD + 1], F32)  # compact AR payload
        ar_sb = persist.tile([128, 4, HD + 1], F32)
        kvr_sb = persist.tile([128, 4, 128], F32R)    # block-diag kv lhsT
        ksb = persist.tile([128, 4, 128], F32R)       # ksum-broadcast lhsT

        with tc.tile_pool(name="p1x", bufs=2) as xpool:
            # issue chunk 0/1 x loads ahead of the weight DMAs
            x_tiles = {}
            xt_handles = {}
            for pre in range(2):
                x_tiles[pre] = xpool.tile([128, 4, D], F32R, tag="x", name="x_sb")
                xcp = x_d[pre * NT:(pre + 1) * NT, :].rearrange(
                    "(t p) f -> p t f", p=128)
                nc.sync.dma_start(out=x_tiles[pre][:, 0:2, :], in_=_r(xcp)[:, 0:2, :])
                nc.sync.dma_start(out=x_tiles[pre][:, 2:4, :], in_=_r(xcp)[:, 2:4, :])

            # weights: k/v columns first (needed earliest), then q, out, bias
            for kc in range(4):
                nc.sync.dma_start(
                    out=wkv_sb[:, kc, :],
                    in_=_r(wqkv_d[kc * 128:(kc + 1) * 128, D:3 * D]),
                )
            for kc in range(4):
                nc.sync.dma_start(
                    out=wq_sb[:, kc, :],
                    in_=_r(wqkv_d[kc * 128:(kc + 1) * 128, 0:D]),
                )
            for kc in range(4):
                nc.sync.dma_start(
                    out=wout_sb[:, kc, :],
                    in_=_r(wout_d[kc * 128:(kc + 1) * 128, :]),
                )
            nc.sync.dma_start(out=bout_sb, in_=bout_d)
            nc.gpsimd.partition_broadcast(bout_full, bout_sb)
            scr = consts.tile([128, D], F32)
            nc.vector.memset(scr, 1.0)
            nc.vector.tensor_copy(ones_col, scr[:, 0:HD])
            # zero the accumulator / block-diag holders off the critical path
            nc.vector.memset(cc_sb, 0.0)
            nc.vector.memset(kvr_sb.bitcast(F32), 0.0)
            nc.vector.memset(ksb.bitcast(F32), 0.0)

            # ---------------- phase 1 ----------------
            def q_prime(ci, xt, qbufs=2):
                """q' = elu(Wq.T x')+1, feature-major, into qp_sb."""
                for c in range(4):
                    q_ps = psq.tile([128, NT], F32, tag="tq", name="q_ps",
                                    bufs=qbufs)
                    for kc in range(4):
                        nc.tensor.matmul(
                            q_ps,
                            wq_sb[:, kc, c * 128:(c + 1) * 128],
                            xt[:, kc, :],
                            start=(kc == 0), stop=(kc == 3),
                        )
                    e2 = small.tile([128, NT], F32, tag="e")
                    nc.scalar.activation(e2, q_ps, AF.Exp)
                    r2 = small.tile([128, NT], F32, tag="r", bufs=1)
                    nc.scalar.activation(r2, q_ps, AF.Relu)
                    nc.vector.scalar_tensor_tensor(
                        qp_sb[:, c, ci * NT:(ci + 1) * NT],
                        e2, 1.0, r2, OP.min, OP.add,
                    )

            with tc.tile_pool(name="p1kp", bufs=1) as kpool, \
                 tc.tile_pool(name="p1v", bufs=1) as vpool, \
                 tc.tile_pool(name="p1s", bufs=2) as small, \
                 tc.tile_pool(name="ps1", bufs=2, space="PSUM") as psum:
                psq = psum  # inline q' shares the phase-1 psum pool
                for ci in range(NCH):
                    if ci not in x_tiles:
                        x_tiles[ci] = xpool.tile([128, 4, D], F32R, tag="x", name="x_sb")
                        xc = x_d[ci * NT:(ci + 1) * NT, :].rearrange(
                            "(t p) f -> p t f", p=128)
                        nc.sync.dma_start(out=x_tiles[ci], in_=_r(xc))
                    x_sb = x_tiles[ci]

                    # transpose x -> x' [fi, n] (fp32r PE transpose)
                    if ci < NCH - DEFER:
                        xt = xpool.tile([128, 4, NT], F32R, tag="xt", bufs=2,
                                        name="xt_work")
                    else:
                        xt = xt_def[:, ci - (NCH - DEFER)]
                    xt_handles[ci] = xt
                    for kc in range(4):
                        tp_ps = psum.tile([128, NT], F32, tag="tq", name="tp_ps", bufs=3)
                        for t in range(4):
                            nc.tensor.transpose(
                                _r(tp_ps[:, t * 128:(t + 1) * 128]),
                                x_sb[:, t, kc * 128:(kc + 1) * 128],
                                ident_sb,
                            )
                        nc.scalar.activation(xt[:, kc, :], tp_ps, AF.Copy)

                    # k, v token-major; elu+1 on k
                    v_sb = vpool.tile([128, 4, VTOT], F32R, tag="v")
                    kp = kpool.tile([128, 4, D], F32R, tag="kp")
                    if ci == 0:
                        # ones columns at c*130+128 (pool has bufs=1, so
                        # writing them once is enough; the per-chunk v
                        # copies never touch these columns)
                        vv = v_sb.rearrange("p t (c w) -> p t c w", w=VW)
                        nc.gpsimd.memset(vv[:, :, 0:4, 128:129].bitcast(F32), 1.0)
                    for t in range(4):
                        k_ps = psum.tile([128, D], F32, tag="k", name="k_ps")
                        v_ps = psum.tile([128, D], F32, tag="vps", name="v_ps", bufs=1)
                        for kc in range(4):
                            st, sp = kc == 0, kc == 3
                            lhsT = xt[:, kc, t * 128:(t + 1) * 128]
                            nc.tensor.matmul(k_ps, lhsT, wkv_sb[:, kc, 0:D],
                                             start=st, stop=sp)
                            nc.tensor.matmul(v_ps, lhsT, wkv_sb[:, kc, D:2 * D],
                                             start=st, stop=sp)
                        # elu(k)+1 = min(exp(k),1) + relu(k)
                        e_sb = small.tile([128, D], F32, tag="e")
                        nc.scalar.activation(e_sb, k_ps, AF.Exp)
                        r_sb = small.tile([128, D], F32, tag="r", bufs=1)
                        nc.scalar.activation(r_sb, k_ps, AF.Relu)
                        nc.vector.scalar_tensor_tensor(
                            kp[:, t, :], e_sb, 1.0, r_sb, OP.min, OP.add)
                        vv = v_sb[:, t, :].rearrange("p (c w) -> p c w", w=VW)
                        nc.scalar.activation(
                            vv[:, 0:4, 0:128],
                            v_ps.rearrange("p (c w) -> p c w", w=128),
                            AF.Copy,
                        )

                    # kv|ksum accumulation per head-pair; rhs spans 260 cols
                    # (neighbor pair data as padding) so fp32r streams at
                    # 1 cyc/row; only cols 0:130 of the result are real.
                    for c in range(4):
                        acc_ps = psum.tile([128, 2 * VW], F32, tag="acc")
                        for t in range(4):
                            nc.tensor.matmul(
                                acc_ps,
                                kp[:, t, c * 128:(c + 1) * 128],
                                v_sb[:, t, c * VW:c * VW + 2 * VW],
                                start=(t == 0), stop=(t == 3),
                            )
                        nc.vector.tensor_add(
                            cc_sb[:, c, :], cc_sb[:, c, :], acc_ps[:, 0:VW])

                    if ci < 1:
                        q_prime(ci, xt, qbufs=3)

            # ---------------- all-reduce kv/ksum (compact payload) --------
            # cmp rows 0:64 = kv_2c, rows 64:128 = kv_2c+1, col 64 = ksum
            nc.gpsimd.tensor_copy(cmp_sb[0:64, :, 0:HD], cc_sb[0:64, :, 0:HD])
            nc.gpsimd.tensor_copy(cmp_sb[64:128, :, 0:HD],
                                  cc_sb[64:128, :, HD:2 * HD])
            nc.gpsimd.tensor_copy(cmp_sb[:, :, HD], cc_sb[:, :, 2 * HD])
            cc_in = dram.tile([128, 4, HD + 1], F32)
            cc_out = dram.tile([128, 4, HD + 1], F32)
            nc.sync.dma_start(out=cc_in, in_=cmp_sb)
            nc.gpsimd.collective_compute(
                "AllReduce", OP.add,
                replica_groups=REPLICA_GROUPS,
                ins=[cc_in.opt()], outs=[cc_out.opt()],
            )

            # ---------------- deferred q' (overlaps the AR) ---------------
            with tc.tile_pool(name="p2s", bufs=2) as small:
                with tc.tile_pool(name="psq", bufs=2, space="PSUM") as psq:
                    for ci in range(1, NCH):
                        q_prime(ci, xt_handles[ci], qbufs=4)

                # AR result -> block-diag lhsTs (emitted after deferred q'
                # so the vector queue isn't head-of-line blocked on the AR)
                nc.sync.dma_start(out=ar_sb, in_=cc_out)
                for h in range(H):
                    po = (h % 2) * 64
                    c = h // 2
                    nc.vector.tensor_scalar_mul(
                        ksb[po:po + 64, c, po:po + 64],
                        ones_col[po:po + 64, :],
                        ar_sb[po:po + 64, c, HD:HD + 1],
                    )
                for c in range(4):
                    nc.vector.tensor_copy(
                        _r(kvr_sb[0:64, c, 0:64]), _r(ar_sb[0:64, c, 0:HD]))
                    nc.vector.tensor_copy(
                        _r(kvr_sb[64:128, c, 64:128]), _r(ar_sb[64:128, c, 0:HD]))

                # ---------------- phase 2 ----------------
                with tc.tile_pool(name="p2y", bufs=2) as ypool, \
                     tc.tile_pool(name="ps2", bufs=2, space="PSUM") as psum2:
                    for ci in range(NCH):
                        # out' and den for both heads of each pair; z-norm
                        # via one DVE divide; result overwrites the dead
                        # qp slot (saves 2MiB of SBUF)
                        for c in range(4):
                            q_rhs = qp_sb[:, c, ci * NT:(ci + 1) * NT]
                            dn_ps = psum2.tile([128, NT], F32, tag="dn", bufs=3)
                            op_ps = psum2.tile([128, NT], F32, tag="op", bufs=3)
                            nc.tensor.matmul(dn_ps, ksb[:, c, :], q_rhs)
                            nc.tensor.matmul(op_ps, kvr_sb[:, c, :], q_rhs)
                            # z = 1/den: approx reciprocal (~18 bits, den is
                            # large & positive so edge cases don't apply);
                            # ost = out'*z overwrites the dead q' slot
                            zr = small.tile([128, NT], F32, tag="zr")
                            nc.vector.reciprocal_approx_fast(out=zr, in_=dn_ps)
                            nc.vector.tensor_mul(q_rhs, op_ps, zr)

                        # y = ost.T @ Wout + bout
                        y_sb = ypool.tile([128, 4, D], F32, tag="y", bufs=1)
                        for t in range(4):
                            y_ps = psum2.tile([128, D], F32, tag="y", bufs=2)
                            for c in range(4):
                                nc.tensor.matmul(
                                    y_ps,
                                    qp_sb[:, c, ci * NT + t * 128:
                                          ci * NT + (t + 1) * 128],
                                    wout_sb[:, c, :],
                                    start=(c == 0), stop=(c == 3),
                                )
                            nc.vector.tensor_add(y_sb[:, t, :], y_ps, bout_full)
                        yc = y_d[ci * NT:(ci + 1) * NT, :].rearrange(
                            "(t p) f -> p t f", p=128)
                        if ci == NCH - 1:
                            nc.sync.dma_start(out=yc[:, 0:2, :], in_=y_sb[:, 0:2, :])
                            nc.scalar.dma_start(out=yc[:, 2:4, :], in_=y_sb[:, 2:4, :])
                        else:
                            nc.sync.dma_start(out=yc, in_=y_sb)


_CACHE = {}


def _get_nc():
    if "nc" in _CACHE:
        return _CACHE["nc"]
    nc = bacc.Bacc(trn_type="TRN2", num_devices=NCORES)
    x_d = nc.dram_tensor("x", [T, D], F32, kind="ExternalInput").ap()
    wqkv_d = nc.dram_tensor("wqkv", [D, 3 * D], F32, kind="ExternalInput").ap()
    wout_d = nc.dram_tensor("wout", [D, D], F32, kind="ExternalInput").ap()
    bout_d = nc.dram_tensor("bout", [1, D], F32, kind="ExternalInput").ap()
    ident_d = nc.dram_tensor("ident", [128, 128], F32, kind="ExternalInput").ap()
    y_d = nc.dram_tensor("y", [T, D], F32, kind="ExternalOutput").ap()
    with tile.TileContext(nc) as tc:
        _build_kernel(tc, nc, x_d, wqkv_d, wout_d, bout_d, ident_d, y_d)
    nc.compile()
    _CACHE["nc"] = nc
    return nc


def kernel(x, Wqkv, Wout, bout, _trace=False, **_trace_kwargs):
    nc = _get_nc()
    x_flat = np.ascontiguousarray(np.asarray(x, dtype=np.float32).reshape(B * N, D))
    wqkv = np.ascontiguousarray(np.asarray(Wqkv, dtype=np.float32))
    wout = np.ascontiguousarray(np.asarray(Wout, dtype=np.float32))
    b = np.ascontiguousarray(np.asarray(bout, dtype=np.float32).reshape(1, D))
    ident = np.eye(128, dtype=np.float32)
    in_maps = []
    for c in range(NCORES):
        shard = np.ascontiguousarray(x_flat[c * T:(c + 1) * T])
        in_maps.append({"x": shard, "wqkv": wqkv, "wout": wout, "bout": b, "ident": ident})
    res = run_bass_kernel_spmd(
        nc, in_maps, core_ids=list(range(NCORES)), trace=_trace, **_trace_kwargs
    )
    y = np.concatenate([res.results[c]["y"] for c in range(NCORES)], axis=0)
    out = y.reshape(B, N, D)
    if _trace:
        return out, res
    return out


# revision 28
# speedup vs baseline: 1.0751x; 1.0751x over previous
"""LinearSelfAttention (elu+1 linear attention) Trainium2 Bass kernel.

Full inputs -> full output. Shards the 32768 tokens (B=4 x N=8192) across 8
NeuronCores as (batch, seq-half); the small kv / k-sum statistics are
all-reduced between the two cores sharing a batch. Weights are replicated.

v2 layout (vs baseline): engineered for tensor-engine continuity (the PE
p-state ramps to 2.4GHz only after ~3us of gapless execution) and for a
hidden AllReduce:

  phase 1 (chunks 0..7): load x chunk [512 tok], PE-transpose to x'
    (feature-major, fp32r transposes: 1.5 cyc/row), k/v projection
    (token-major), elu+1 on k via scalar-Exp + scalar-Relu + DVE
    min/add combine (Exp/Relu/Copy live in one act table -> zero
    table reloads), kv|ksum accumulated per head-pair with an
    overlapping-stride rhs (N=260 >= 256 keeps fp32r at 1 cyc/row).
    Chunks 0..3 also compute q' (feature-major) inline.
  AllReduce: only the diagonal kv blocks + ksum ship (133KB not 266KB);
    chunks 4..7 defer their q' work until after the AR is dispatched so
    the PE keeps streaming through the collective.
  phase 2 (chunks 0..7): block-diagonal kv / ksum-broadcast matmuls give
    out'[e,n] and den[e,n]; z-normalize via a single DVE divide
    (no Ln/Exp round trip); y = out'.T @ Wout with the bias added by
    gpsimd during the PSUM->SBUF copy (bout pre-broadcast to 128
    partitions). ost overwrites the dead qp slot to save SBUF.

All matmul operands are float32r (fp22-rounded fp32) - 1 cyc/row for
free size >= 256, ~1e-4 relative error. fp32r matmuls require even N
and outputs at base partition 0 (walrus ISA constraints).
"""

import numpy as np

import concourse.bass as bass
import concourse.bacc as bacc
import concourse.mybir as mybir
import concourse.tile as tile
from concourse.bass_utils import run_bass_kernel_spmd

B, N, D, H, HD = 4, 8192, 512, 8, 64
NCORES = 8
T = B * N // NCORES          # 4096 tokens per core
NT = 512                     # tokens per chunk
NCH = T // NT                # 8 chunks
DEFER = 5                    # chunks NCH-DEFER..NCH-1 defer q' past the AR
VW = 130                     # per-pair rhs stride: [v_2c|v_2c+1 (128) | 1 | pad]
VTOT = 5 * VW                # 650: pair c reads cols c*130 .. c*130+260
F32 = mybir.dt.float32
F32R = mybir.dt.float32r
AF = mybir.ActivationFunctionType
OP = mybir.AluOpType

REPLICA_GROUPS = [[0, 1], [2, 3], [4, 5], [6, 7]]


def _r(ap):
    return ap.bitcast(F32R)


def _build_kernel(tc, nc, x_d, wqkv_d, wout_d, bout_d, ident_d, y_d):
    with (
        tc.tile_pool(name="consts", bufs=1) as consts,
        tc.tile_pool(name="persist", bufs=1) as persist,
        tc.tile_pool(name="dram", bufs=1, space="DRAM") as dram,
    ):
        # ---------------- constants (DMA priority: ident, x0 ... weights) ---
        ident_sb = consts.tile([128, 128], F32R)
        nc.sync.dma_start(out=ident_sb, in_=_r(ident_d))

        # x chunk pool hoisted so chunk 0's load is issued before weights
        wkv_sb = consts.tile([128, 4, 2 * D], F32R)   # Wqkv cols D:3D
        wq_sb = consts.tile([128, 4, D], F32R)        # Wqkv cols 0:D
        wout_sb = consts.tile([128, 4, D], F32R)
        bout_sb = consts.tile([1, D], F32)
        bout_full = consts.tile([128, D], F32)
        ones_col = consts.tile([128, HD], F32R)

        # persistent state
        qp_sb = persist.tile([128, 4, T], F32R)       # q' (feature-major)
        xt_def = persist.tile([128, DEFER, 4, NT], F32R)  # x' for deferred q'
        cc_sb = persist.tile([128, 4, VW], F32)       # kv|ksum accumulator
        cmp_sb = persist.tile([128, 4, HD + 1], F32)  # compact AR payload
        ar_sb = persist.tile([128, 4, HD + 1], F32)
        kvr_sb = persist.tile([128, 4, 128], F32R)    # block-diag kv lhsT
        ksb = persist.tile([128, 4, 128], F32R)       # ksum-broadcast lhsT

        with tc.tile_pool(name="p1x", bufs=2) as xpool:
            # issue chunk 0's x load ahead of the weight DMAs
            x_tiles = {}
            x_tiles[0] = xpool.tile([128, 4, D], F32R, tag="x", name="x_sb")
            xc0 = x_d[0:NT, :].rearrange("(t p) f -> p t f", p=128)
            nc.sync.dma_start(out=x_tiles[0][:, 0:2, :], in_=_r(xc0)[:, 0:2, :])
            nc.sync.dma_start(out=x_tiles[0][:, 2:4, :], in_=_r(xc0)[:, 2:4, :])

            # weights: k/v columns first (needed earliest), then q, out, bias
            for kc in range(4):
                nc.sync.dma_start(
                    out=wkv_sb[:, kc, :],
                    in_=_r(wqkv_d[kc * 128:(kc + 1) * 128, D:3 * D]),
                )
            for kc in range(4):
                nc.sync.dma_start(
                    out=wq_sb[:, kc, :],
                    in_=_r(wqkv_d[kc * 128:(kc + 1) * 128, 0:D]),
                )
            for kc in range(4):
                nc.sync.dma_start(
                    out=wout_sb[:, kc, :],
                    in_=_r(wout_d[kc * 128:(kc + 1) * 128, :]),
                )
            nc.sync.dma_start(out=bout_sb, in_=bout_d)
            nc.gpsimd.partition_broadcast(bout_full, bout_sb)
            scr = consts.tile([128, D], F32)
            nc.vector.memset(scr, 1.0)
            nc.vector.tensor_copy(ones_col, scr[:, 0:HD])
            # zero the accumulator / block-diag holders off the critical path
            nc.vector.memset(cc_sb, 0.0)
            nc.vector.memset(kvr_sb.bitcast(F32), 0.0)
            nc.vector.memset(ksb.bitcast(F32), 0.0)

            # ---------------- phase 1 ----------------
            def q_prime(ci, xt, qbufs=2):
                """q' = elu(Wq.T x')+1, feature-major, into qp_sb."""
                for c in range(4):
                    q_ps = psq.tile([128, NT], F32, tag="tq", name="q_ps",
                                    bufs=qbufs)
                    for kc in range(4):
                        nc.tensor.matmul(
                            q_ps,
                            wq_sb[:, kc, c * 128:(c + 1) * 128],
                            xt[:, kc, :],
                            start=(kc == 0), stop=(kc == 3),
                        )
                    e2 = small.tile([128, NT], F32, tag="e")
                    nc.scalar.activation(e2, q_ps, AF.Exp)
                    r2 = small.tile([128, NT], F32, tag="r", bufs=1)
                    nc.scalar.activation(r2, q_ps, AF.Relu)
                    nc.vector.scalar_tensor_tensor(
                        qp_sb[:, c, ci * NT:(ci + 1) * NT],
                        e2, 1.0, r2, OP.min, OP.add,
                    )

            with tc.tile_pool(name="p1w", bufs=2) as work, \
                 tc.tile_pool(name="p1kp", bufs=1) as kpool, \
                 tc.tile_pool(name="p1v", bufs=1) as vpool, \
                 tc.tile_pool(name="p1s", bufs=2) as small, \
                 tc.tile_pool(name="ps1", bufs=2, space="PSUM") as psum:
                psq = psum  # inline q' shares the phase-1 psum pool
                for ci in range(NCH):
                    if ci not in x_tiles:
                        x_tiles[ci] = xpool.tile([128, 4, D], F32R, tag="x", name="x_sb")
                        xc = x_d[ci * NT:(ci + 1) * NT, :].rearrange(
                            "(t p) f -> p t f", p=128)
                        nc.sync.dma_start(out=x_tiles[ci], in_=_r(xc))
                    x_sb = x_tiles[ci]

                    # transpose x -> x' [fi, n] (fp32r PE transpose)
                    if ci < NCH - DEFER:
                        xt = work.tile([128, 4, NT], F32R, tag="xt")
                    else:
                        xt = xt_def[:, ci - (NCH - DEFER)]
                    for kc in range(4):
                        tp_ps = psum.tile([128, NT], F32, tag="tq", name="tp_ps", bufs=3)
                        for t in range(4):
                            nc.tensor.transpose(
                                _r(tp_ps[:, t * 128:(t + 1) * 128]),
                                x_sb[:, t, kc * 128:(kc + 1) * 128],
                                ident_sb,
                            )
                        nc.scalar.activation(xt[:, kc, :], tp_ps, AF.Copy)

                    # k, v token-major; elu+1 on k
                    v_sb = vpool.tile([128, 4, VTOT], F32R, tag="v")
                    kp = kpool.tile([128, 4, D], F32R, tag="kp")
                    if ci == 0:
                        # ones columns at c*130+128 (pool has bufs=1, so
                        # writing them once is enough; the per-chunk v
                        # copies never touch these columns)
                        vv = v_sb.rearrange("p t (c w) -> p t c w", w=VW)
                        nc.gpsimd.memset(vv[:, :, 0:4, 128:129].bitcast(F32), 1.0)
                    for t in range(4):
                        k_ps = psum.tile([128, D], F32, tag="k", name="k_ps")
                        v_ps = psum.tile([128, D], F32, tag="vps", name="v_ps", bufs=1)
                        for kc in range(4):
                            st, sp = kc == 0, kc == 3
                            lhsT = xt[:, kc, t * 128:(t + 1) * 128]
                            nc.tensor.matmul(k_ps, lhsT, wkv_sb[:, kc, 0:D],
                                             start=st, stop=sp)
                            nc.tensor.matmul(v_ps, lhsT, wkv_sb[:, kc, D:2 * D],
                                             start=st, stop=sp)
                        # elu(k)+1 = min(exp(k),1) + relu(k)
                        e_sb = small.tile([128, D], F32, tag="e")
                        nc.scalar.activation(e_sb, k_ps, AF.Exp)
                        r_sb = small.tile([128, D], F32, tag="r", bufs=1)
                        nc.scalar.activation(r_sb, k_ps, AF.Relu)
                        nc.vector.scalar_tensor_tensor(
                            kp[:, t, :], e_sb, 1.0, r_sb, OP.min, OP.add)
                        vv = v_sb[:, t, :].rearrange("p (c w) -> p c w", w=VW)
                        nc.scalar.activation(
                            vv[:, 0:4, 0:128],
                            v_ps.rearrange("p (c w) -> p c w", w=128),
                            AF.Copy,
                        )

                    # kv|ksum accumulation per head-pair; rhs spans 260 cols
                    # (neighbor pair data as padding) so fp32r streams at
                    # 1 cyc/row; only cols 0:130 of the result are real.
                    for c in range(4):
                        acc_ps = psum.tile([128, 2 * VW], F32, tag="acc")
                        for t in range(4):
                            nc.tensor.matmul(
                                acc_ps,
                                kp[:, t, c * 128:(c + 1) * 128],
                                v_sb[:, t, c * VW:c * VW + 2 * VW],
                                start=(t == 0), stop=(t == 3),
                            )
                        nc.vector.tensor_add(
                            cc_sb[:, c, :], cc_sb[:, c, :], acc_ps[:, 0:VW])

                    if ci < NCH - DEFER:
                        q_prime(ci, xt, qbufs=3)

            # ---------------- all-reduce kv/ksum (compact payload) --------
            # cmp rows 0:64 = kv_2c, rows 64:128 = kv_2c+1, col 64 = ksum
            nc.gpsimd.tensor_copy(cmp_sb[0:64, :, 0:HD], cc_sb[0:64, :, 0:HD])
            nc.gpsimd.tensor_copy(cmp_sb[64:128, :, 0:HD],
                                  cc_sb[64:128, :, HD:2 * HD])
            nc.gpsimd.tensor_copy(cmp_sb[:, :, HD], cc_sb[:, :, 2 * HD])
            cc_in = dram.tile([128, 4, HD + 1], F32)
            cc_out = dram.tile([128, 4, HD + 1], F32)
            nc.sync.dma_start(out=cc_in, in_=cmp_sb)
            nc.gpsimd.collective_compute(
                "AllReduce", OP.add,
                replica_groups=REPLICA_GROUPS,
                ins=[cc_in.opt()], outs=[cc_out.opt()],
            )

            # ---------------- deferred q' (overlaps the AR) ---------------
            with tc.tile_pool(name="p2s", bufs=2) as small:
                with tc.tile_pool(name="psq", bufs=2, space="PSUM") as psq:
                    for ci in range(NCH - DEFER, NCH):
                        q_prime(ci, xt_def[:, ci - (NCH - DEFER)], qbufs=4)

                # AR result -> block-diag lhsTs (emitted after deferred q'
                # so the vector queue isn't head-of-line blocked on the AR)
                nc.sync.dma_start(out=ar_sb, in_=cc_out)
                for h in range(H):
                    po = (h % 2) * 64
                    c = h // 2
                    nc.vector.tensor_scalar_mul(
                        ksb[po:po + 64, c, po:po + 64],
                        ones_col[po:po + 64, :],
                        ar_sb[po:po + 64, c, HD:HD + 1],
                    )
                for c in range(4):
                    nc.vector.tensor_copy(
                        _r(kvr_sb[0:64, c, 0:64]), _r(ar_sb[0:64, c, 0:HD]))
                    nc.vector.tensor_copy(
                        _r(kvr_sb[64:128, c, 64:128]), _r(ar_sb[64:128, c, 0:HD]))

                # ---------------- phase 2 ----------------
                with tc.tile_pool(name="p2y", bufs=2) as ypool, \
                     tc.tile_pool(name="ps2", bufs=2, space="PSUM") as psum2:
                    for ci in range(NCH):
                        # out' and den for both heads of each pair; z-norm
                        # via one DVE divide; result overwrites the dead
                        # qp slot (saves 2MiB of SBUF)
                        for c in range(4):
                            q_rhs = qp_sb[:, c, ci * NT:(ci + 1) * NT]
                            dn_ps = psum2.tile([128, NT], F32, tag="dn", bufs=3)
                            op_ps = psum2.tile([128, NT], F32, tag="op", bufs=3)
                            nc.tensor.matmul(dn_ps, ksb[:, c, :], q_rhs)
                            nc.tensor.matmul(op_ps, kvr_sb[:, c, :], q_rhs)
                            # z = 1/den: approx reciprocal (~18 bits, den is
                            # large & positive so edge cases don't apply);
                            # ost = out'*z overwrites the dead q' slot
                            zr = small.tile([128, NT], F32, tag="zr")
                            nc.vector.reciprocal_approx_fast(out=zr, in_=dn_ps)
                            nc.vector.tensor_mul(q_rhs, op_ps, zr)

                        # y = ost.T @ Wout + bout
                        y_sb = ypool.tile([128, 4, D], F32, tag="y")
                        for t in range(4):
                            y_ps = psum2.tile([128, D], F32, tag="y", bufs=2)
                            for c in range(4):
                                nc.tensor.matmul(
                                    y_ps,
                                    qp_sb[:, c, ci * NT + t * 128:
                                          ci * NT + (t + 1) * 128],
                                    wout_sb[:, c, :],
                                    start=(c == 0), stop=(c == 3),
                                )
                            nc.vector.tensor_add(y_sb[:, t, :], y_ps, bout_full)
                        yc = y_d[ci * NT:(ci + 1) * NT, :].rearrange(
                            "(t p) f -> p t f", p=128)
                        if ci == NCH - 1:
                            nc.sync.dma_start(out=yc[:, 0:2, :], in_=y_sb[:, 0:2, :])
                            nc.scalar.dma_start(out=yc[:, 2:4, :], in_=y_sb[:, 2:4, :])
                        else:
                            nc.sync.dma_start(out=yc, in_=y_sb)


_CACHE = {}


def _get_nc():
    if "nc" in _CACHE:
        return _CACHE["nc"]
    nc = bacc.Bacc(trn_type="TRN2", num_devices=NCORES)
    x_d = nc.dram_tensor("x", [T, D], F32, kind="ExternalInput").ap()
    wqkv_d = nc.dram_tensor("wqkv", [D, 3 * D], F32, kind="ExternalInput").ap()
    wout_d = nc.dram_tensor("wout", [D, D], F32, kind="ExternalInput").ap()
    bout_d = nc.dram_tensor("bout", [1, D], F32, kind="ExternalInput").ap()
    ident_d = nc.dram_tensor("ident", [128, 128], F32, kind="ExternalInput").ap()
    y_d = nc.dram_tensor("y", [T, D], F32, kind="ExternalOutput").ap()
    with tile.TileContext(nc) as tc:
        _build_kernel(tc, nc, x_d, wqkv_d, wout_d, bout_d, ident_d, y_d)
    nc.compile()
    _CACHE["nc"] = nc
    return nc


def kernel(x, Wqkv, Wout, bout, _trace=False, **_trace_kwargs):
    nc = _get_nc()
    x_flat = np.ascontiguousarray(np.asarray(x, dtype=np.float32).reshape(B * N, D))
    wqkv = np.ascontiguousarray(np.asarray(Wqkv, dtype=np.float32))
    wout = np.ascontiguousarray(np.asarray(Wout, dtype=np.float32))
    b = np.ascontiguousarray(np.asarray(bout, dtype=np.float32).reshape(1, D))
    ident = np.eye(128, dtype=np.float32)
    in_maps = []
    for c in range(NCORES):
        shard = np.ascontiguousarray(x_flat[c * T:(c + 1) * T])
        in_maps.append({"x": shard, "wqkv": wqkv, "wout": wout, "bout": b, "ident": ident})
    res = run_bass_kernel_spmd(
        nc, in_maps, core_ids=list(range(NCORES)), trace=_trace, **_trace_kwargs
    )
    y = np.concatenate([res.results[c]["y"] for c in range(NCORES)], axis=0)
    out = y.reshape(B, N, D)
    if _trace:
        return out, res
    return out


# revision 29
# speedup vs baseline: 1.0875x; 1.0115x over previous
"""LinearSelfAttention (elu+1 linear attention) Trainium2 Bass kernel.

Full inputs -> full output. Shards the 32768 tokens (B=4 x N=8192) across 8
NeuronCores as (batch, seq-half); the small kv / k-sum statistics are
all-reduced between the two cores sharing a batch. Weights are replicated.

v2 layout (vs baseline): engineered for tensor-engine continuity (the PE
p-state ramps to 2.4GHz only after ~3us of gapless execution) and for a
hidden AllReduce:

  phase 1 (chunks 0..7): load x chunk [512 tok], PE-transpose to x'
    (feature-major, fp32r transposes: 1.5 cyc/row), k/v projection
    (token-major), elu+1 on k via scalar-Exp + scalar-Relu + DVE
    min/add combine (Exp/Relu/Copy live in one act table -> zero
    table reloads), kv|ksum accumulated per head-pair with an
    overlapping-stride rhs (N=260 >= 256 keeps fp32r at 1 cyc/row).
    Chunks 0..3 also compute q' (feature-major) inline.
  AllReduce: only the diagonal kv blocks + ksum ship (133KB not 266KB);
    chunks 4..7 defer their q' work until after the AR is dispatched so
    the PE keeps streaming through the collective.
  phase 2 (chunks 0..7): block-diagonal kv / ksum-broadcast matmuls give
    out'[e,n] and den[e,n]; z-normalize via a single DVE divide
    (no Ln/Exp round trip); y = out'.T @ Wout with the bias added by
    gpsimd during the PSUM->SBUF copy (bout pre-broadcast to 128
    partitions). ost overwrites the dead qp slot to save SBUF.

All matmul operands are float32r (fp22-rounded fp32) - 1 cyc/row for
free size >= 256, ~1e-4 relative error. fp32r matmuls require even N
and outputs at base partition 0 (walrus ISA constraints).
"""

import numpy as np

import concourse.bass as bass
import concourse.bacc as bacc
import concourse.mybir as mybir
import concourse.tile as tile
from concourse.bass_utils import run_bass_kernel_spmd

B, N, D, H, HD = 4, 8192, 512, 8, 64
NCORES = 8
T = B * N // NCORES          # 4096 tokens per core
NT = 512                     # tokens per chunk
NCH = T // NT                # 8 chunks
DEFER = 5                    # chunks NCH-DEFER..NCH-1 defer q' past the AR
VW = 130                     # per-pair rhs stride: [v_2c|v_2c+1 (128) | 1 | pad]
VTOT = 5 * VW                # 650: pair c reads cols c*130 .. c*130+260
F32 = mybir.dt.float32
F32R = mybir.dt.float32r
AF = mybir.ActivationFunctionType
OP = mybir.AluOpType

REPLICA_GROUPS = [[0, 1], [2, 3], [4, 5], [6, 7]]


def _r(ap):
    return ap.bitcast(F32R)


def _build_kernel(tc, nc, x_d, wqkv_d, wout_d, bout_d, ident_d, y_d):
    with (
        tc.tile_pool(name="consts", bufs=1) as consts,
        tc.tile_pool(name="persist", bufs=1) as persist,
        tc.tile_pool(name="dram", bufs=1, space="DRAM") as dram,
    ):
        # ---------------- constants (DMA priority: ident, x0 ... weights) ---
        ident_sb = consts.tile([128, 128], F32R)
        nc.sync.dma_start(out=ident_sb, in_=_r(ident_d))

        # x chunk pool hoisted so chunk 0's load is issued before weights
        wkv_sb = consts.tile([128, 4, 2 * D], F32R)   # Wqkv cols D:3D
        wq_sb = consts.tile([128, 4, D], F32R)        # Wqkv cols 0:D
        wout_sb = consts.tile([128, 4, D], F32R)
        bout_sb = consts.tile([1, D], F32)
        bout_full = consts.tile([128, D], F32)
        ones_col = consts.tile([128, HD], F32R)

        # persistent state
        qp_sb = persist.tile([128, 4, T], F32R)       # q' (feature-major)
        xt_def = persist.tile([128, DEFER, 4, NT], F32R)  # x' for deferred q'
        cc_sb = persist.tile([128, 4, VW], F32)       # kv|ksum accumulator
        cmp_sb = persist.tile([128, 4, HD + 1], F32)  # compact AR payload
        ar_sb = persist.tile([128, 4, HD + 1], F32)
        kvr_sb = persist.tile([128, 4, 128], F32R)    # block-diag kv lhsT
        ksb = persist.tile([128, 4, 128], F32R)       # ksum-broadcast lhsT

        with tc.tile_pool(name="p1x", bufs=2) as xpool:
            # issue chunk 0's x load ahead of the weight DMAs
            x_tiles = {}
            x_tiles[0] = xpool.tile([128, 4, D], F32R, tag="x", name="x_sb")
            xc0 = x_d[0:NT, :].rearrange("(t p) f -> p t f", p=128)
            nc.sync.dma_start(out=x_tiles[0][:, 0:2, :], in_=_r(xc0)[:, 0:2, :])
            nc.sync.dma_start(out=x_tiles[0][:, 2:4, :], in_=_r(xc0)[:, 2:4, :])

            # weights: k/v columns first (needed earliest), then q, out, bias
            for kc in range(4):
                nc.sync.dma_start(
                    out=wkv_sb[:, kc, :],
                    in_=_r(wqkv_d[kc * 128:(kc + 1) * 128, D:3 * D]),
                )
            for kc in range(4):
                nc.sync.dma_start(
                    out=wq_sb[:, kc, :],
                    in_=_r(wqkv_d[kc * 128:(kc + 1) * 128, 0:D]),
                )
            for kc in range(4):
                nc.sync.dma_start(
                    out=wout_sb[:, kc, :],
                    in_=_r(wout_d[kc * 128:(kc + 1) * 128, :]),
                )
            nc.sync.dma_start(out=bout_sb, in_=bout_d)
            nc.gpsimd.partition_broadcast(bout_full, bout_sb)
            scr = consts.tile([128, D], F32)
            nc.vector.memset(scr, 1.0)
            nc.vector.tensor_copy(ones_col, scr[:, 0:HD])
            # zero the accumulator / block-diag holders off the critical path
            nc.vector.memset(cc_sb, 0.0)
            nc.vector.memset(kvr_sb.bitcast(F32), 0.0)
            nc.vector.memset(ksb.bitcast(F32), 0.0)

            # ---------------- phase 1 ----------------
            def q_prime(ci, xt, qbufs=2):
                """q' = elu(Wq.T x')+1, feature-major, into qp_sb."""
                for c in range(4):
                    q_ps = psq.tile([128, NT], F32, tag="tq", name="q_ps",
                                    bufs=qbufs)
                    for kc in range(4):
                        nc.tensor.matmul(
                            q_ps,
                            wq_sb[:, kc, c * 128:(c + 1) * 128],
                            xt[:, kc, :],
                            start=(kc == 0), stop=(kc == 3),
                        )
                    e2 = small.tile([128, NT], F32, tag="e")
                    nc.scalar.activation(e2, q_ps, AF.Exp)
                    r2 = small.tile([128, NT], F32, tag="r", bufs=1)
                    nc.scalar.activation(r2, q_ps, AF.Relu)
                    nc.vector.scalar_tensor_tensor(
                        qp_sb[:, c, ci * NT:(ci + 1) * NT],
                        e2, 1.0, r2, OP.min, OP.add,
                    )

            with tc.tile_pool(name="p1w", bufs=2) as work, \
                 tc.tile_pool(name="p1kp", bufs=1) as kpool, \
                 tc.tile_pool(name="p1v", bufs=1) as vpool, \
                 tc.tile_pool(name="p1s", bufs=2) as small, \
                 tc.tile_pool(name="ps1", bufs=2, space="PSUM") as psum:
                psq = psum  # inline q' shares the phase-1 psum pool
                for ci in range(NCH):
                    if ci not in x_tiles:
                        x_tiles[ci] = xpool.tile([128, 4, D], F32R, tag="x", name="x_sb")
                        xc = x_d[ci * NT:(ci + 1) * NT, :].rearrange(
                            "(t p) f -> p t f", p=128)
                        nc.sync.dma_start(out=x_tiles[ci], in_=_r(xc))
                    x_sb = x_tiles[ci]

                    # transpose x -> x' [fi, n] (fp32r PE transpose)
                    if ci < NCH - DEFER:
                        xt = work.tile([128, 4, NT], F32R, tag="xt")
                    else:
                        xt = xt_def[:, ci - (NCH - DEFER)]
                    for kc in range(4):
                        tp_ps = psum.tile([128, NT], F32, tag="tq", name="tp_ps", bufs=3)
                        for t in range(4):
                            nc.tensor.transpose(
                                _r(tp_ps[:, t * 128:(t + 1) * 128]),
                                x_sb[:, t, kc * 128:(kc + 1) * 128],
                                ident_sb,
                            )
                        nc.scalar.activation(xt[:, kc, :], tp_ps, AF.Copy)

                    # k, v token-major; elu+1 on k
                    v_sb = vpool.tile([128, 4, VTOT], F32R, tag="v")
                    kp = kpool.tile([128, 4, D], F32R, tag="kp")
                    if ci == 0:
                        # ones columns at c*130+128 (pool has bufs=1, so
                        # writing them once is enough; the per-chunk v
                        # copies never touch these columns)
                        vv = v_sb.rearrange("p t (c w) -> p t c w", w=VW)
                        nc.gpsimd.memset(vv[:, :, 0:4, 128:129].bitcast(F32), 1.0)
                    for t in range(4):
                        k_ps = psum.tile([128, D], F32, tag="k", name="k_ps")
                        v_ps = psum.tile([128, D], F32, tag="vps", name="v_ps", bufs=1)
                        for kc in range(4):
                            st, sp = kc == 0, kc == 3
                            lhsT = xt[:, kc, t * 128:(t + 1) * 128]
                            nc.tensor.matmul(k_ps, lhsT, wkv_sb[:, kc, 0:D],
                                             start=st, stop=sp)
                            nc.tensor.matmul(v_ps, lhsT, wkv_sb[:, kc, D:2 * D],
                                             start=st, stop=sp)
                        # elu(k)+1 = min(exp(k),1) + relu(k)
                        e_sb = small.tile([128, D], F32, tag="e")
                        nc.scalar.activation(e_sb, k_ps, AF.Exp)
                        r_sb = small.tile([128, D], F32, tag="r", bufs=1)
                        nc.scalar.activation(r_sb, k_ps, AF.Relu)
                        nc.vector.scalar_tensor_tensor(
                            kp[:, t, :], e_sb, 1.0, r_sb, OP.min, OP.add)
                        vv = v_sb[:, t, :].rearrange("p (c w) -> p c w", w=VW)
                        nc.scalar.activation(
                            vv[:, 0:4, 0:128],
                            v_ps.rearrange("p (c w) -> p c w", w=128),
                            AF.Copy,
                        )

                    # kv|ksum accumulation per head-pair; rhs spans 260 cols
                    # (neighbor pair data as padding) so fp32r streams at
                    # 1 cyc/row; only cols 0:130 of the result are real.
                    for cp in range(2):
                        ca, cb = 2 * cp, 2 * cp + 1
                        acc_a = psum.tile([128, 2 * VW], F32, tag="acc",
                                          name="acc_a")
                        acc_b = psum.tile([128, 2 * VW], F32, tag="acc",
                                          name="acc_b")
                        for t in range(4):
                            nc.tensor.matmul(
                                acc_a,
                                kp[:, t, ca * 128:(ca + 1) * 128],
                                v_sb[:, t, ca * VW:ca * VW + 2 * VW],
                                start=(t == 0), stop=(t == 3),
                            )
                            nc.tensor.matmul(
                                acc_b,
                                kp[:, t, cb * 128:(cb + 1) * 128],
                                v_sb[:, t, cb * VW:cb * VW + 2 * VW],
                                start=(t == 0), stop=(t == 3),
                            )
                        nc.vector.tensor_add(
                            cc_sb[:, ca, :], cc_sb[:, ca, :], acc_a[:, 0:VW])
                        nc.vector.tensor_add(
                            cc_sb[:, cb, :], cc_sb[:, cb, :], acc_b[:, 0:VW])

                    if ci < NCH - DEFER:
                        q_prime(ci, xt, qbufs=3)

            # ---------------- all-reduce kv/ksum (compact payload) --------
            # cmp rows 0:64 = kv_2c, rows 64:128 = kv_2c+1, col 64 = ksum
            nc.gpsimd.tensor_copy(cmp_sb[0:64, :, 0:HD], cc_sb[0:64, :, 0:HD])
            nc.gpsimd.tensor_copy(cmp_sb[64:128, :, 0:HD],
                                  cc_sb[64:128, :, HD:2 * HD])
            nc.gpsimd.tensor_copy(cmp_sb[:, :, HD], cc_sb[:, :, 2 * HD])
            cc_in = dram.tile([128, 4, HD + 1], F32)
            cc_out = dram.tile([128, 4, HD + 1], F32)
            nc.sync.dma_start(out=cc_in, in_=cmp_sb)
            nc.gpsimd.collective_compute(
                "AllReduce", OP.add,
                replica_groups=REPLICA_GROUPS,
                ins=[cc_in.opt()], outs=[cc_out.opt()],
            )

            # ---------------- deferred q' (overlaps the AR) ---------------
            with tc.tile_pool(name="p2s", bufs=2) as small:
                with tc.tile_pool(name="psq", bufs=2, space="PSUM") as psq:
                    for ci in range(NCH - DEFER, NCH):
                        q_prime(ci, xt_def[:, ci - (NCH - DEFER)], qbufs=4)

                # AR result -> block-diag lhsTs (emitted after deferred q'
                # so the vector queue isn't head-of-line blocked on the AR)
                nc.sync.dma_start(out=ar_sb, in_=cc_out)
                for h in range(H):
                    po = (h % 2) * 64
                    c = h // 2
                    nc.vector.tensor_scalar_mul(
                        ksb[po:po + 64, c, po:po + 64],
                        ones_col[po:po + 64, :],
                        ar_sb[po:po + 64, c, HD:HD + 1],
                    )
                for c in range(4):
                    nc.vector.tensor_copy(
                        _r(kvr_sb[0:64, c, 0:64]), _r(ar_sb[0:64, c, 0:HD]))
                    nc.vector.tensor_copy(
                        _r(kvr_sb[64:128, c, 64:128]), _r(ar_sb[64:128, c, 0:HD]))

                # ---------------- phase 2 ----------------
                with tc.tile_pool(name="p2y", bufs=2) as ypool, \
                     tc.tile_pool(name="ps2", bufs=2, space="PSUM") as psum2:
                    for ci in range(NCH):
                        # out' and den for both heads of each pair; z-norm
                        # via one DVE divide; result overwrites the dead
                        # qp slot (saves 2MiB of SBUF)
                        for c in range(4):
                            q_rhs = qp_sb[:, c, ci * NT:(ci + 1) * NT]
                            dn_ps = psum2.tile([128, NT], F32, tag="dn", bufs=3)
                            op_ps = psum2.tile([128, NT], F32, tag="op", bufs=3)
                            nc.tensor.matmul(dn_ps, ksb[:, c, :], q_rhs)
                            nc.tensor.matmul(op_ps, kvr_sb[:, c, :], q_rhs)
                            # z = 1/den: approx reciprocal (~18 bits, den is
                            # large & positive so edge cases don't apply);
                            # ost = out'*z overwrites the dead q' slot
                            zr = small.tile([128, NT], F32, tag="zr")
                            nc.vector.reciprocal_approx_fast(out=zr, in_=dn_ps)
                            nc.vector.tensor_mul(q_rhs, op_ps, zr)

                        # y = ost.T @ Wout + bout (t-pairs interleaved
                        # across the two y PSUM banks: ~15% faster streams)
                        y_sb = ypool.tile([128, 4, D], F32, tag="y")
                        for tp in range(2):
                            ta, tb = 2 * tp, 2 * tp + 1
                            y_pa = psum2.tile([128, D], F32, tag="y", bufs=2,
                                              name="y_pa")
                            y_pb = psum2.tile([128, D], F32, tag="y", bufs=2,
                                              name="y_pb")
                            for c in range(4):
                                nc.tensor.matmul(
                                    y_pa,
                                    qp_sb[:, c, ci * NT + ta * 128:
                                          ci * NT + (ta + 1) * 128],
                                    wout_sb[:, c, :],
                                    start=(c == 0), stop=(c == 3),
                                )
                                nc.tensor.matmul(
                                    y_pb,
                                    qp_sb[:, c, ci * NT + tb * 128:
                                          ci * NT + (tb + 1) * 128],
                                    wout_sb[:, c, :],
                                    start=(c == 0), stop=(c == 3),
                                )
                            nc.vector.tensor_add(y_sb[:, ta, :], y_pa, bout_full)
                            nc.vector.tensor_add(y_sb[:, tb, :], y_pb, bout_full)
                        yc = y_d[ci * NT:(ci + 1) * NT, :].rearrange(
                            "(t p) f -> p t f", p=128)
                        if ci == NCH - 1:
                            nc.sync.dma_start(out=yc[:, 0:2, :], in_=y_sb[:, 0:2, :])
                            nc.scalar.dma_start(out=yc[:, 2:4, :], in_=y_sb[:, 2:4, :])
                        else:
                            nc.sync.dma_start(out=yc, in_=y_sb)


_CACHE = {}


def _get_nc():
    if "nc" in _CACHE:
        return _CACHE["nc"]
    nc = bacc.Bacc(trn_type="TRN2", num_devices=NCORES)
    x_d = nc.dram_tensor("x", [T, D], F32, kind="ExternalInput").ap()
    wqkv_d = nc.dram_tensor("wqkv", [D, 3 * D], F32, kind="ExternalInput").ap()
    wout_d = nc.dram_tensor("wout", [D, D], F32, kind="ExternalInput").ap()
    bout_d = nc.dram_tensor("bout", [1, D], F32, kind="ExternalInput").ap()
    ident_d = nc.dram_tensor("ident", [128, 128], F32, kind="ExternalInput").ap()
    y_d = nc.dram_tensor("y", [T, D], F32, kind="ExternalOutput").ap()
    with tile.TileContext(nc) as tc:
        _build_kernel(tc, nc, x_d, wqkv_d, wout_d, bout_d, ident_d, y_d)
    nc.compile()
    _CACHE["nc"] = nc
    return nc


def kernel(x, Wqkv, Wout, bout, _trace=False, **_trace_kwargs):
    nc = _get_nc()
    x_flat = np.ascontiguousarray(np.asarray(x, dtype=np.float32).reshape(B * N, D))
    wqkv = np.ascontiguousarray(np.asarray(Wqkv, dtype=np.float32))
    wout = np.ascontiguousarray(np.asarray(Wout, dtype=np.float32))
    b = np.ascontiguousarray(np.asarray(bout, dtype=np.float32).reshape(1, D))
    ident = np.eye(128, dtype=np.float32)
    in_maps = []
    for c in range(NCORES):
        shard = np.ascontiguousarray(x_flat[c * T:(c + 1) * T])
        in_maps.append({"x": shard, "wqkv": wqkv, "wout": wout, "bout": b, "ident": ident})
    res = run_bass_kernel_spmd(
        nc, in_maps, core_ids=list(range(NCORES)), trace=_trace, **_trace_kwargs
    )
    y = np.concatenate([res.results[c]["y"] for c in range(NCORES)], axis=0)
    out = y.reshape(B, N, D)
    if _trace:
        return out, res
    return out
